# Initial kernel scaffold
#
"""Self-contained Bass/Tile GAT kernel for TRN2 (8-core SPMD).

kernel(**inputs) takes the FULL unsharded inputs (as produced by
setup_inputs) and returns the FULL [50000, 10] float32 output.


Table layout (bf16 units):
  table0 row: [h 256 bf16 | as0 4 f32 as 8 bf16-units]  = 264 units (528B)
  table1 row: [h1 256 bf16 | as1 1 f32 as 2 units]      = 258 units (516B)
Attention 'as' columns stay f32 (bitcast view); 'ad' vectors are bf16.
All matmuls bf16 x bf16 -> f32 PSUM.
"""
import numpy as np
import concourse.bass as bass
import concourse.bacc as bacc
import concourse.tile as tile
import concourse.mybir as mybir
import ml_dtypes

F32 = mybir.dt.float32
BF16 = mybir.dt.bfloat16
I32 = mybir.dt.int32
AX = mybir.AluOpType
AF = mybir.ActivationFunctionType

NCORE = 8
BLK = 128
BPC = 49
VLOC = BPC * BLK            # 6272
NPAD = NCORE * VLOC         # 50176
H0 = 4
T0U = 256 + 2 * H0          # 264 bf16 units per table0 row
T1U = 256 + 2               # 258 bf16 units per table1 row
SLOPE = 0.2
CHB = [0, 2048, 3584, 5120, 6272]   # AG chunk row boundaries (own rows)
NCHUNK = len(CHB) - 1
MLPG = 4


def build_nc(KT):
    TPC = BPC * KT
    nc = bacc.Bacc("TRN2", target_bir_lowering=False, debug=False, num_devices=NCORE)

    xT = nc.dram_tensor("xT", [128, NPAD], BF16, kind="ExternalInput")
    x_ownT = nc.dram_tensor("x_ownT", [128, VLOC], BF16, kind="ExternalInput")
    srcidx0 = nc.dram_tensor("srcidx0", [128, TPC], I32, kind="ExternalInput")
    srcidx1 = nc.dram_tensor("srcidx1", [128, TPC], I32, kind="ExternalInput")
    dstloc = nc.dram_tensor("dstloc", [128, TPC], BF16, kind="ExternalInput")
    W0cat = nc.dram_tensor("W0cat", [128, 264], BF16, kind="ExternalInput")  # h|as|ad
    W1catP = nc.dram_tensor("W1catP", [128, 2 * 258], BF16, kind="ExternalInput")
    lw1P = nc.dram_tensor("lw1P", [128, 1024], BF16, kind="ExternalInput")
    lw2P = nc.dram_tensor("lw2P", [128, 4096], BF16, kind="ExternalInput")
    lw3P = nc.dram_tensor("lw3P", [128, 80], BF16, kind="ExternalInput")
    lb1c = nc.dram_tensor("lb1c", [128, 4], F32, kind="ExternalInput")
    lb2c = nc.dram_tensor("lb2c", [128, 8], F32, kind="ExternalInput")
    lb3b = nc.dram_tensor("lb3b", [128, 10], F32, kind="ExternalInput")
    b0b = nc.dram_tensor("b0b", [128, 256], F32, kind="ExternalInput")
    b1b = nc.dram_tensor("b1b", [128, 256], F32, kind="ExternalInput")
    iotaf = nc.dram_tensor("iotaf", [128, 128], BF16, kind="ExternalInput")
    ident = nc.dram_tensor("ident", [128, 128], BF16, kind="ExternalInput")
    onesrow = nc.dram_tensor("onesrow", [1, 128], BF16, kind="ExternalInput")
    iotacol = nc.dram_tensor("iotacol", [128, 1], F32, kind="ExternalInput")
    dstlocF = nc.dram_tensor("dstlocF", [BPC, KT * 128], BF16, kind="ExternalInput")

    out = nc.dram_tensor("out", [VLOC, 10], F32, kind="ExternalOutput")

    table0 = nc.dram_tensor("table0", [NPAD, T0U], BF16)
    ag_in = nc.dram_tensor("ag_in", [VLOC, T1U], BF16)
    table1 = nc.dram_tensor("table1", [NPAD, T1U], BF16, addr_space="Shared")

    with tile.TileContext(nc) as tc:
        with (
            tc.tile_pool(name="const", bufs=1) as cpool,
            tc.tile_pool(name="xt", bufs=4) as xpool,
            tc.tile_pool(name="tw", bufs=4) as twpool,
            tc.tile_pool(name="g", bufs=12) as gpool,
            tc.tile_pool(name="st", bufs=4) as stpool,
            tc.tile_pool(name="s", bufs=3) as spool,
            tc.tile_pool(name="w", bufs=8) as wpool,
            tc.tile_pool(name="rhs", bufs=4) as rpool,
            tc.tile_pool(name="act", bufs=4) as apool,
            tc.tile_pool(name="h2T", bufs=4) as hpool,
            tc.tile_pool(name="a1T", bufs=8) as a1pool,
            tc.tile_pool(name="a2T", bufs=10) as a2pool,
            tc.tile_pool(name="ps_big", bufs=2, space="PSUM") as ps_big,
            tc.tile_pool(name="ps_mlp", bufs=2, space="PSUM") as ps_mlp,
            tc.tile_pool(name="ps_agg", bufs=2, space="PSUM") as ps_agg,
            tc.tile_pool(name="ps_se", bufs=2, space="PSUM") as ps_se,
        ):
            def cload(src, shape, dt):
                t = cpool.tile(shape, dt, tag=src.name + "_sb")
                nc.sync.dma_start(out=t[:], in_=src[:])
                return t

            W0c_sb = cload(W0cat, [128, 264], BF16)
            W1c_sb = cload(W1catP, [128, 2 * 258], BF16)
            lw1_sb = cload(lw1P, [128, 1024], BF16)
            lw2_sb = cload(lw2P, [128, 4096], BF16)
            lw3_sb = cload(lw3P, [128, 80], BF16)
            lb1_sb = cload(lb1c, [128, 4], F32)
            lb2_sb = cload(lb2c, [128, 8], F32)
            lb3_sb = cload(lb3b, [128, 10], F32)
            b0b_sb = cload(b0b, [128, 256], F32)
            b1b_sb = cload(b1b, [128, 256], F32)
            iota_sb = cload(iotaf, [128, 128], BF16)
            id_sb = cload(ident, [128, 128], BF16)
            ones_sb = cload(onesrow, [1, 128], BF16)
            iocol_sb = cload(iotacol, [128, 1], F32)
            si0_sb = cload(srcidx0, [128, TPC], I32)
            si1_sb = cload(srcidx1, [128, TPC], I32)
            dl_sb = cload(dstloc, [128, TPC], BF16)
            ad0_sb = cpool.tile([128, BPC * H0], BF16, tag="ad0")
            ad1_sb = cpool.tile([128, BPC], BF16, tag="ad1")

            # ---- phase T0: replicated table0 = xp @ W0cat, 4 blocks/iter ----
            TB = 8
            for it in range(NPAD // (128 * TB)):
                xt = xpool.tile([128, 128 * TB], BF16, tag="xt")
                nc.sync.dma_start(
                    out=xt[:], in_=xT[:, it * 128 * TB:(it + 1) * 128 * TB])
                tw = twpool.tile([128, TB * T0U], BF16, tag="tw")
                for k in range(TB):
                    ps = ps_big.tile([128, 260], F32, tag="psb")
                    nc.tensor.matmul(ps[:], xt[:, k * 128:(k + 1) * 128],
                                     W0c_sb[:, 0:260], start=True, stop=True)
                    nc.vector.tensor_copy(
                        out=tw[:, k * T0U:k * T0U + 256], in_=ps[:, 0:256])
                    nc.scalar.copy(
                        out=tw[:, k * T0U + 256:(k + 1) * T0U].bitcast(F32),
                        in_=ps[:, 256:260])
                nc.scalar.dma_start(
                    out=table0[it * 128 * TB:(it + 1) * 128 * TB, :].rearrange(
                        "(b p) u -> p b u", p=128),
                    in_=tw[:].rearrange("p (b u) -> p b u", u=T0U))

            # ---- phase T0-own: ad0 (bf16) for own nodes ----
            for it in range(BPC // TB + 1):
                blo = it * TB
                bhi = min(blo + TB, BPC)
                if blo >= bhi:
                    break
                nb = bhi - blo
                xt = xpool.tile([128, 128 * TB], BF16, tag="xt")
                nc.sync.dma_start(out=xt[:, :128 * nb],
                                  in_=x_ownT[:, blo * 128:bhi * 128])
                for k in range(nb):
                    ps = ps_se.tile([128, H0], F32, tag="pse")
                    nc.tensor.matmul(ps[:], xt[:, k * 128:(k + 1) * 128],
                                     W0c_sb[:, 260:264], start=True, stop=True)
                    nc.scalar.copy(
                        out=ad0_sb[:, (blo + k) * H0:(blo + k + 1) * H0], in_=ps[:])

            # ---- edge block (shared) ----
            NQ4 = KT * 128 // 4   # S_wide built in 4 pieces

            def edge_block(b, table, tunits, H, ad_ap, si_sb):
                # S_wide[n, e] = (dstloc[e] == n) for the whole block, built by
                # broadcasting dstloc down partitions via a K=1 ones-matmul,
                # then comparing against the per-partition index.
                s_wide = spool.tile([128, KT * 128], BF16, tag="s")
                dlf = wpool.tile([1, KT * 128], BF16, tag="dlf")
                nc.sync.dma_start(out=dlf[:], in_=dstlocF[b:b + 1, :])
                for q in range(4):
                    psq = ps_big.tile([128, NQ4], F32, tag="psb")
                    nc.tensor.matmul(psq[:], ones_sb[:],
                                     dlf[:, q * NQ4:(q + 1) * NQ4],
                                     start=True, stop=True)
                    nc.vector.tensor_tensor(
                        out=s_wide[:, q * NQ4:(q + 1) * NQ4],
                        in0=psq[:],
                        in1=iocol_sb[:].to_broadcast([128, NQ4]),
                        op=AX.is_equal)
                agg = ps_agg.tile([128, 256 + H0], F32, tag="agg")
                aggv = agg[:, :256 + H]
                for t in range(KT):
                    col = b * KT + t
                    g = gpool.tile([128, T0U], BF16, tag="g")
                    gv = g[:, :tunits]
                    nc.gpsimd.indirect_dma_start(
                        out=gv, out_offset=None, in_=table[:],
                        in_offset=bass.IndirectOffsetOnAxis(
                            ap=si_sb[:, col:col + 1], axis=0),
                    )
                    g_as = g[:, 256:256 + 2 * H].bitcast(F32)       # [128, H] f32
                    st = stpool.tile([128, 128], BF16, tag="st")
                    nc.vector.tensor_tensor(
                        out=st[:],
                        in0=dl_sb[:, col:col + 1].to_broadcast([128, 128]),
                        in1=iota_sb[:], op=AX.is_equal)
                    se = ps_se.tile([128, H0], F32, tag="pse")
                    sev = se[:, :H]
                    nc.tensor.matmul(sev, s_wide[:, t * 128:(t + 1) * 128],
                                     ad_ap, start=True, stop=True)
                    sfull = wpool.tile([128, H0], F32, tag="sfull")
                    nc.vector.tensor_tensor(out=sfull[:, :H], in0=sev,
                                            in1=g_as, op=AX.add)
                    w1 = wpool.tile([128, H0], F32, tag="w1")
                    nc.scalar.activation(w1[:, :H], sfull[:, :H], AF.Exp)
                    w2 = wpool.tile([128, H0], F32, tag="w2")
                    nc.scalar.activation(w2[:, :H], sfull[:, :H], AF.Exp, scale=SLOPE)
                    rhs = rpool.tile([128, 256 + H0], BF16, tag="rhs")
                    if H == 1:
                        wf = wpool.tile([128, 1], F32, tag="wf")
                        nc.vector.tensor_tensor(out=wf[:], in0=w1[:, :1],
                                                in1=w2[:, :1], op=AX.max)
                        nc.vector.tensor_copy(out=rhs[:, 256:257], in_=wf[:])
                        nc.vector.tensor_scalar_mul(
                            rhs[:, 0:256], g[:, 0:256], wf[:])
                    else:
                        nc.vector.tensor_tensor(out=rhs[:, 256:256 + H], in0=w1[:, :H],
                                                in1=w2[:, :H], op=AX.max)
                        nc.vector.tensor_tensor(
                            out=rhs[:, 0:256].rearrange("p (a b) -> p a b", b=H),
                            in0=g[:, 0:256].rearrange("p (a b) -> p a b", b=H),
                            in1=rhs[:, 256:256 + H].unsqueeze(1).to_broadcast([128, 64, H]),
                            op=AX.mult)
                    nc.tensor.matmul(aggv, st[:], rhs[:, :256 + H],
                                     start=(t == 0), stop=(t == KT - 1))
                return agg

            def finish_block(agg, H, bias_sb):
                dn = wpool.tile([128, H0], F32, tag="dn")
                nc.scalar.activation(dn[:, :H], agg[:, 256:256 + H], AF.Copy,
                                     bias=1e-30)
                rc = wpool.tile([128, H0], F32, tag="rc")
                nc.vector.reciprocal(rc[:, :H], dn[:, :H])
                h = apool.tile([128, 256], F32, tag="h")
                if H == 1:
                    nc.vector.tensor_scalar_mul(h[:], agg[:, 0:256], rc[:, 0:1])
                else:
                    nc.vector.tensor_tensor(
                        out=h[:].rearrange("p (a b) -> p a b", b=H),
                        in0=agg[:, 0:256].rearrange("p (a b) -> p a b", b=H),
                        in1=rc[:, :H].unsqueeze(1).to_broadcast([128, 64, H]),
                        op=AX.mult)
                nc.vector.tensor_add(h[:], h[:], bias_sb[:])
                hr = apool.tile([128, 256], BF16, tag="hr")
                nc.scalar.activation(hr[:], h[:], AF.Relu)
                return hr

            # ---- phase E0 + table1 (+ chunked AllGather) ----
            for b in range(BPC):
                agg = edge_block(b, table0, T0U, H0,
                                 ad0_sb[:, b * H0:(b + 1) * H0], si0_sb)
                hr = finish_block(agg, H0, b0b_sb)
                ps1 = ps_big.tile([128, 259], F32, tag="psb")
                for k in range(2):
                    tps = ps_mlp.tile([128, 128], BF16, tag="psmlp")
                    nc.tensor.transpose(tps[:], hr[:, k * 128:(k + 1) * 128], id_sb[:])
                    c = spool.tile([128, 128], BF16, tag="hT")
                    nc.vector.tensor_copy(out=c[:], in_=tps[:])
                    nc.tensor.matmul(ps1[:, :258], c[:],
                                     W1c_sb[:, k * 258:(k + 1) * 258],
                                     start=(k == 0), stop=(k == 1))
                t1 = twpool.tile([128, T1U], BF16, tag="tw")
                nc.vector.tensor_copy(out=t1[:, 0:256], in_=ps1[:, 0:256])
                nc.scalar.copy(out=t1[:, 256:258].bitcast(F32), in_=ps1[:, 256:257])
                nc.scalar.copy(out=ad1_sb[:, b:b + 1], in_=ps1[:, 257:258])
                nc.sync.dma_start(out=ag_in[b * 128:(b + 1) * 128, :], in_=t1[:])
                for cch in range(NCHUNK):
                    ready_block = CHB[cch + 1] // BLK - 1
                    if b == ready_block:
                        nc.gpsimd.collective_compute(
                            "AllGather", AX.bypass,
                            replica_groups=[list(range(NCORE))],
                            ins=[ag_in[CHB[cch]:CHB[cch + 1], :]],
                            outs=[table1[NCORE * CHB[cch]:
                                         NCORE * CHB[cch + 1], :]],
                        )

            # ---- phase E1 + batched MLP ----
            gstart = 0
            while gstart < BPC:
                G = min(MLPG, BPC - gstart)
                h2T0 = hpool.tile([128, 128 * MLPG], BF16, tag="h2T0")
                h2T1 = hpool.tile([128, 128 * MLPG], BF16, tag="h2T1")
                for gi in range(G):
                    b = gstart + gi
                    agg = edge_block(b, table1, T1U, 1, ad1_sb[:, b:b + 1], si1_sb)
                    h2 = finish_block(agg, 1, b1b_sb)
                    for k, wt in enumerate((h2T0, h2T1)):
                        tps = ps_mlp.tile([128, 128], BF16, tag="psmlp")
                        nc.tensor.transpose(tps[:], h2[:, k * 128:(k + 1) * 128], id_sb[:])
                        nc.vector.tensor_copy(out=wt[:, gi * 128:(gi + 1) * 128], in_=tps[:])
                NW = 128 * G
                h2T = (h2T0, h2T1)
                a1T = []
                for oc in range(4):
                    ps = ps_mlp.tile([128, 128 * MLPG], F32, tag="psmlp")
                    for kc in range(2):
                        nc.tensor.matmul(
                            ps[:, :NW],
                            lw1_sb[:, kc * 512 + oc * 128:kc * 512 + (oc + 1) * 128],
                            h2T[kc][:, :NW], start=(kc == 0), stop=(kc == 1))
                    a = a1pool.tile([128, 128 * MLPG], BF16, tag="a1T")
                    nc.scalar.activation(a[:, :NW], ps[:, :NW], AF.Relu,
                                         bias=lb1_sb[:, oc:oc + 1])
                    a1T.append(a)
                a2T = []
                for oc in range(8):
                    ps = ps_mlp.tile([128, 128 * MLPG], F32, tag="psmlp")
                    for kc in range(4):
                        nc.tensor.matmul(
                            ps[:, :NW],
                            lw2_sb[:, kc * 1024 + oc * 128:kc * 1024 + (oc + 1) * 128],
                            a1T[kc][:, :NW], start=(kc == 0), stop=(kc == 3))
                    a = a2pool.tile([128, 128 * MLPG], BF16, tag="a2T")
                    nc.scalar.activation(a[:, :NW], ps[:, :NW], AF.Relu,
                                         bias=lb2_sb[:, oc:oc + 1])
                    a2T.append(a)
                for gi in range(G):
                    b = gstart + gi
                    ps3 = ps_se.tile([128, 16], F32, tag="pse")
                    for kc in range(8):
                        nc.tensor.matmul(ps3[:, :10],
                                         a2T[kc][:, gi * 128:(gi + 1) * 128],
                                         lw3_sb[:, kc * 10:(kc + 1) * 10],
                                         start=(kc == 0), stop=(kc == 7))
                    lg = wpool.tile([128, 16], F32, tag="lg")
                    nc.vector.tensor_add(lg[:, :10], ps3[:, :10], lb3_sb[:])
                    elg = wpool.tile([128, 16], F32, tag="elg")
                    nc.scalar.activation(elg[:, :10], lg[:, :10], AF.Relu)
                    nc.scalar.activation(elg[:, :10], elg[:, :10], AF.Exp)
                    ssum = wpool.tile([128, 1], F32, tag="ssum")
                    nc.vector.reduce_sum(out=ssum[:], in_=elg[:, :10],
                                         axis=mybir.AxisListType.X)
                    rcs = wpool.tile([128, 1], F32, tag="rcs")
                    nc.vector.reciprocal(rcs[:], ssum[:])
                    ob = wpool.tile([128, 16], F32, tag="ob")
                    nc.vector.tensor_scalar_mul(ob[:, :10], elg[:, :10], rcs[:])
                    nc.sync.dma_start(out=out[b * 128:(b + 1) * 128, :], in_=ob[:, :10])
                gstart += G

    nc.compile()
    return nc


# ======== host-side prep ========
import heapq

def build_graph(edge_index):
    src, dst = edge_index[0].astype(np.int64), edge_index[1].astype(np.int64)
    keep = src != dst
    src, dst = src[keep], dst[keep]
    loops = np.arange(50000, dtype=np.int64)
    return np.concatenate([src, loops]), np.concatenate([dst, loops])


def assign_blocks(dst, EPB):
    N = 50000
    deg = np.bincount(dst, minlength=N)
    order = np.argsort(-deg, kind='stable')
    nblocks = NCORE * BPC
    heap = [(0, b) for b in range(nblocks)]
    heapq.heapify(heap)
    slots_used = np.zeros(nblocks, np.int64)
    load = np.zeros(nblocks, np.int64)
    block_of = np.empty(N, np.int64)
    deferred = []
    for v in order:
        d = deg[v]
        while True:
            if not heap:
                return None, None, load
            l, b = heapq.heappop(heap)
            if slots_used[b] < BLK and load[b] + d <= EPB:
                block_of[v] = b
                slots_used[b] += 1
                load[b] += d
                heapq.heappush(heap, (load[b], b))
                break
            deferred.append(b)
        for bb in deferred:
            if slots_used[bb] < BLK:
                heapq.heappush(heap, (load[bb], bb))
        deferred.clear()
    perm = np.full(NPAD, -1, np.int64)
    newid = np.empty(N, np.int64)
    fill = np.zeros(nblocks, np.int64)
    for v in range(N):
        b = block_of[v]
        s = b * BLK + fill[b]
        fill[b] += 1
        perm[s] = v
        newid[v] = s
    return perm, newid, load


def chunk_rowmap():
    s = np.arange(NPAD)
    core = s // VLOC
    pos = s % VLOC
    chb = np.asarray(CHB)
    ch = np.searchsorted(chb, pos, side='right') - 1
    width = chb[ch + 1] - chb[ch]
    return NCORE * chb[ch] + core * width + (pos - chb[ch])


def build_streams(src, dst, newid, KT):
    EPB = KT * 128
    nsrc, ndst = newid[src], newid[dst]
    blk = ndst // BLK
    order = np.argsort(blk, kind='stable')
    nsrc_o, ndst_o, blk_o = nsrc[order], ndst[order], blk[order]
    counts = np.bincount(blk_o, minlength=NCORE * BPC)
    assert counts.max() <= EPB
    starts = np.concatenate([[0], np.cumsum(counts)[:-1]])
    within = np.arange(len(blk_o)) - starts[blk_o]
    cmap = chunk_rowmap()
    src0 = np.zeros((NCORE * BPC, EPB), np.int32)
    src1 = np.zeros((NCORE * BPC, EPB), np.int32)
    dstloc = np.full((NCORE * BPC, EPB), -1.0, np.float32)
    src0[blk_o, within] = nsrc_o.astype(np.int32)
    src1[blk_o, within] = cmap[nsrc_o].astype(np.int32)
    dstloc[blk_o, within] = (ndst_o % BLK).astype(np.float32)
    src0 = src0.reshape(NCORE, BPC * KT, 128)
    src1 = src1.reshape(NCORE, BPC * KT, 128)
    dstloc = dstloc.reshape(NCORE, BPC * KT, 128)
    s0 = [np.ascontiguousarray(src0[c].T) for c in range(NCORE)]
    s1 = [np.ascontiguousarray(src1[c].T) for c in range(NCORE)]
    ds = [np.ascontiguousarray(dstloc[c].T.astype(ml_dtypes.bfloat16)) for c in range(NCORE)]
    dsF = [np.ascontiguousarray(dstloc[c].reshape(BPC, KT * 128).astype(ml_dtypes.bfloat16))
           for c in range(NCORE)]
    return s0, s1, ds, dsF


def head_perm_cols():
    j = np.arange(256)
    return (j % H0) * 64 + j // H0


def prep_inputs(inputs, KT):
    bf = ml_dtypes.bfloat16
    x = np.asarray(inputs['x'], np.float32)
    src, dst = build_graph(np.asarray(inputs['edge_index']))
    perm, newid, load = assign_blocks(dst, KT * 128)
    assert perm is not None, f"packing failed at KT={KT}, max load {load.max()}"
    s0, s1, ds, dsF = build_streams(src, dst, newid, KT)
    sig = head_perm_cols()
    W0 = np.asarray(inputs['W0'], np.float32)
    W1 = np.asarray(inputs['W1'], np.float32)
    a_src0 = np.asarray(inputs['a_src0'], np.float32)
    a_dst0 = np.asarray(inputs['a_dst0'], np.float32)
    a_src1 = np.asarray(inputs['a_src1'], np.float32).reshape(256, 1)
    a_dst1 = np.asarray(inputs['a_dst1'], np.float32).reshape(256, 1)
    W0p = W0[:, sig]
    jj = np.arange(256)
    cc, hh = jj // H0, jj % H0
    A0s = np.zeros((256, H0), np.float32)
    A0d = np.zeros((256, H0), np.float32)
    A0s[jj, hh] = a_src0[hh, cc]
    A0d[jj, hh] = a_dst0[hh, cc]
    W0cat = np.concatenate([W0p, W0p @ A0s, W0p @ A0d], 1).astype(bf)   # [128, 264]
    W1p = W1[sig, :]
    W1cat = np.concatenate([W1p, W1p @ a_src1, W1p @ a_dst1], 1)        # [256, 258]
    W1catP = np.ascontiguousarray(
        W1cat.reshape(2, 128, 258).transpose(1, 0, 2).reshape(128, 2 * 258)).astype(bf)
    lw1 = np.asarray(inputs['lw1'], np.float32)
    lw2 = np.asarray(inputs['lw2'], np.float32)
    lw3 = np.asarray(inputs['lw3'], np.float32)
    lw1P = np.ascontiguousarray(lw1.reshape(2, 128, 512).transpose(1, 0, 2).reshape(128, 1024)).astype(bf)
    lw2P = np.ascontiguousarray(lw2.reshape(4, 128, 1024).transpose(1, 0, 2).reshape(128, 4096)).astype(bf)
    lw3P = np.ascontiguousarray(lw3.reshape(8, 128, 10).transpose(1, 0, 2).reshape(128, 80)).astype(bf)
    lb1c = np.ascontiguousarray(np.asarray(inputs['lb1'], np.float32).reshape(4, 128).T)
    lb2c = np.ascontiguousarray(np.asarray(inputs['lb2'], np.float32).reshape(8, 128).T)
    lb3b = np.ascontiguousarray(np.tile(np.asarray(inputs['lb3'], np.float32), (128, 1)))
    b0p = np.asarray(inputs['b0'], np.float32)[sig]
    b0b = np.tile(b0p, (128, 1)).astype(np.float32)
    b1b = np.tile(np.asarray(inputs['b1'], np.float32), (128, 1)).astype(np.float32)
    iotaf = np.tile(np.arange(128, dtype=np.float32), (128, 1)).astype(bf)
    ident = np.eye(128, dtype=np.float32).astype(bf)
    onesrow = np.ones((1, 128), dtype=np.float32).astype(bf)
    iotacol = np.arange(128, dtype=np.float32).reshape(128, 1)

    real = perm >= 0
    xp = np.zeros((NPAD, 128), np.float32)
    xp[real] = x[perm[real]]
    xT_full = np.ascontiguousarray(xp.T).astype(bf)

    common = dict(xT=xT_full, W0cat=W0cat, W1catP=W1catP, lw1P=lw1P, lw2P=lw2P,
                  lw3P=lw3P, lb1c=lb1c, lb2c=lb2c, lb3b=lb3b, b0b=b0b, b1b=b1b,
                  iotaf=iotaf, ident=ident, onesrow=onesrow, iotacol=iotacol)
    in_maps = []
    for c in range(NCORE):
        m = dict(common)
        m['x_ownT'] = np.ascontiguousarray(xp[c * VLOC:(c + 1) * VLOC].T).astype(bf)
        m['srcidx0'] = s0[c]
        m['srcidx1'] = s1[c]
        m['dstloc'] = ds[c]
        m['dstlocF'] = dsF[c]
        in_maps.append(m)
    return in_maps, perm


def pick_KT(inputs):
    src, dst = build_graph(np.asarray(inputs['edge_index']))
    for KT in (9, 10, 11):
        perm, newid, load = assign_blocks(dst, KT * 128)
        if perm is not None:
            return KT
    raise RuntimeError("packing failed")


def assemble_output(results, perm):
    full = np.concatenate([results[c]['out'] for c in range(NCORE)], 0)
    out = np.zeros((50000, 10), np.float32)
    real = perm >= 0
    out[perm[real]] = full[real]
    return out


# ======== public entrypoint ========
_CACHE = {}


def _get_nc(KT):
    if ('nc', KT) not in _CACHE:
        _CACHE[('nc', KT)] = build_nc(KT)
    return _CACHE[('nc', KT)]


def run(inputs, trace=False, trace_cores=None, tmpdir=None):
    from concourse.bass_utils import run_bass_kernel_spmd
    KT = pick_KT(inputs)
    nc = _get_nc(KT)
    in_maps, perm = prep_inputs(inputs, KT)
    res = run_bass_kernel_spmd(nc, in_maps, list(range(NCORE)),
                               trace=trace, trace_cores=trace_cores,
                               tmpdir=tmpdir)
    return assemble_output(res.results, perm), res


def kernel(**inputs):
    out, _ = run(inputs, trace=False)
    return out



# revision 30
# speedup vs baseline: 1.0404x; 1.0404x over previous
"""Self-contained Bass/Tile GAT kernel for TRN2 (8-core SPMD).

kernel(**inputs) takes the FULL unsharded inputs (as produced by
setup_inputs) and returns the FULL [50000, 10] float32 output.

Layout (bf16 units, head-major h storage):
  table0 row: [h 256 bf16 (head-major) | as0 4 f32 as 8 bf16-units] = 264 units
  table1 row: [h1 256 bf16 | as1 1 f32 as 2 units]                 = 258 units
Edge one-hot matrices (st: [e,n] scatter, sw: [n,e] gather) are
host-precomputed constants streamed from DRAM per block.
Per block: ONE indirect gather for all KT tiles; attention softmax math
batched across tiles with strided 3D/4D APs.
"""
import numpy as np
import concourse.bass as bass
import concourse.bacc as bacc
import concourse.tile as tile
import concourse.mybir as mybir
import ml_dtypes

F32 = mybir.dt.float32
BF16 = mybir.dt.bfloat16
I32 = mybir.dt.int32
AX = mybir.AluOpType
AF = mybir.ActivationFunctionType

NCORE = 8
BLK = 128
BPC = 49
VLOC = BPC * BLK            # 6272
NPAD = NCORE * VLOC         # 50176
H0 = 4
T0U = 256 + 2 * H0          # 264 bf16 units per table0 row
T1U = 256 + 2               # 258 bf16 units per table1 row
SLOPE = 0.2
CHB = [0, 1536, 3072, 4352, 5248, 5888, 6272]   # AG chunk row boundaries (own rows)
NCHUNK = len(CHB) - 1
MLPG = 4


def build_nc(KT):
    TPC = BPC * KT
    nc = bacc.Bacc("TRN2", target_bir_lowering=False, debug=False, num_devices=NCORE)

    xT = nc.dram_tensor("xT", [128, NPAD], BF16, kind="ExternalInput")
    x_ownT = nc.dram_tensor("x_ownT", [128, VLOC], BF16, kind="ExternalInput")
    srcidx0 = nc.dram_tensor("srcidx0", [128, TPC], I32, kind="ExternalInput")
    srcidx1 = nc.dram_tensor("srcidx1", [128, TPC], I32, kind="ExternalInput")
    stD = nc.dram_tensor("stD", [128, TPC * 128], BF16, kind="ExternalInput")
    swD = nc.dram_tensor("swD", [128, TPC * 128], BF16, kind="ExternalInput")
    W0cat = nc.dram_tensor("W0cat", [128, 264], BF16, kind="ExternalInput")  # h|as|ad
    W1catP = nc.dram_tensor("W1catP", [128, 2 * 258], BF16, kind="ExternalInput")
    lw1P = nc.dram_tensor("lw1P", [128, 1024], BF16, kind="ExternalInput")
    lw2P = nc.dram_tensor("lw2P", [128, 4096], BF16, kind="ExternalInput")
    lw3P = nc.dram_tensor("lw3P", [128, 80], BF16, kind="ExternalInput")
    lb1c = nc.dram_tensor("lb1c", [128, 4], F32, kind="ExternalInput")
    lb2c = nc.dram_tensor("lb2c", [128, 8], F32, kind="ExternalInput")
    lb3col = nc.dram_tensor("lb3col", [128, 1], F32, kind="ExternalInput")
    b0b = nc.dram_tensor("b0b", [128, 256], F32, kind="ExternalInput")
    b1b = nc.dram_tensor("b1b", [128, 256], F32, kind="ExternalInput")
    ident = nc.dram_tensor("ident", [128, 128], BF16, kind="ExternalInput")
    identf = nc.dram_tensor("identf", [16, 16], F32, kind="ExternalInput")

    out = nc.dram_tensor("out", [VLOC, 10], F32, kind="ExternalOutput")

    table0 = nc.dram_tensor("table0", [NPAD, T0U], BF16)
    t0own = nc.dram_tensor("t0own", [VLOC, T0U], BF16)
    ag_in = nc.dram_tensor("ag_in", [VLOC, T1U], BF16)
    table1 = nc.dram_tensor("table1", [NPAD, T1U], BF16, addr_space="Shared")

    with tile.TileContext(nc) as tc:
        with (
            tc.tile_pool(name="const", bufs=1) as cpool,
            tc.tile_pool(name="xt", bufs=3) as xpool,
            tc.tile_pool(name="tw", bufs=3) as twpool,
            tc.tile_pool(name="g", bufs=8) as gpool,
            tc.tile_pool(name="stp", bufs=5) as stpool,
            tc.tile_pool(name="swp", bufs=5) as swpool,
            tc.tile_pool(name="w", bufs=12) as wpool,
            tc.tile_pool(name="rhs", bufs=5) as rpool,
            tc.tile_pool(name="act", bufs=4) as apool,
            tc.tile_pool(name="h2T", bufs=4) as hpool,
            tc.tile_pool(name="a1T", bufs=8) as a1pool,
            tc.tile_pool(name="a2T", bufs=10) as a2pool,
            tc.tile_pool(name="ps_big", bufs=2, space="PSUM") as ps_big,
            tc.tile_pool(name="ps_mlp", bufs=2, space="PSUM") as ps_mlp,
            tc.tile_pool(name="ps_agg", bufs=2, space="PSUM") as ps_agg,
            tc.tile_pool(name="ps_se", bufs=2, space="PSUM") as ps_se,
        ):
            def cload(src, shape, dt):
                t = cpool.tile(shape, dt, tag=src.name + "_sb")
                nc.sync.dma_start(out=t[:], in_=src[:])
                return t

            W0c_sb = cload(W0cat, [128, 264], BF16)
            W1c_sb = cload(W1catP, [128, 2 * 258], BF16)
            lw1_sb = cload(lw1P, [128, 1024], BF16)
            lw2_sb = cload(lw2P, [128, 4096], BF16)
            lw3_sb = cload(lw3P, [128, 80], BF16)
            lb1_sb = cload(lb1c, [128, 4], F32)
            lb2_sb = cload(lb2c, [128, 8], F32)
            lb3_sb = cload(lb3col, [128, 1], F32)
            b0b_sb = cload(b0b, [128, 256], F32)
            b1b_sb = cload(b1b, [128, 256], F32)
            id_sb = cload(ident, [128, 128], BF16)
            idf_sb = cload(identf, [16, 16], F32)
            si0_sb = cload(srcidx0, [128, TPC], I32)
            si1_sb = cload(srcidx1, [128, TPC], I32)
            ad0_sb = cpool.tile([128, BPC * H0], BF16, tag="ad0")
            ad1_sb = cpool.tile([128, BPC], BF16, tag="ad1")

            # ---- phase T0-own: per-core rows [h|as] -> t0own, plus ad0 ----
            TB = 8
            TBM = 16
            for it in range(BPC // TB + 1):
                blo = it * TB
                bhi = min(blo + TB, BPC)
                if blo >= bhi:
                    break
                nb = bhi - blo
                xt = xpool.tile([128, 128 * TB], BF16, tag="xt")
                nc.sync.dma_start(out=xt[:, :128 * nb],
                                  in_=x_ownT[:, blo * 128:bhi * 128])
                two = twpool.tile([128, TB * T0U], BF16, tag="two")
                for k in range(nb):
                    ps = ps_big.tile([128, 264], F32, tag="psb")
                    nc.tensor.matmul(ps[:], xt[:, k * 128:(k + 1) * 128],
                                     W0c_sb[:, 0:264], start=True, stop=True)
                    nc.vector.tensor_copy(
                        out=two[:, k * T0U:k * T0U + 256], in_=ps[:, 0:256])
                    nc.scalar.copy(
                        out=two[:, k * T0U + 256:(k + 1) * T0U].bitcast(F32),
                        in_=ps[:, 256:260])
                    nc.scalar.copy(
                        out=ad0_sb[:, (blo + k) * H0:(blo + k + 1) * H0],
                        in_=ps[:, 260:264])
                nc.sync.dma_start(
                    out=t0own[blo * 128:bhi * 128, :].rearrange(
                        "(b p) u -> p b u", p=128),
                    in_=two[:, :nb * T0U].rearrange("p (b u) -> p b u", u=T0U))

            # ---- phase T0: replicated table0 = xp @ W0cat, 8 blocks/iter;
            #      copies alternate vector/scalar to balance engines ----
            NBLK = NPAD // 128
            for it in range((NBLK + TBM - 1) // TBM):
                blo = it * TBM
                nbm = min(TBM, NBLK - blo)
                xt = xpool.tile([128, 128 * TBM], BF16, tag="xtm")
                nc.sync.dma_start(
                    out=xt[:, :128 * nbm],
                    in_=xT[:, blo * 128:(blo + nbm) * 128])
                tw = twpool.tile([128, TBM * T0U], BF16, tag="twm")
                tw3 = tw[:].rearrange("p (b u) -> p b u", u=T0U)
                for k in range(nbm):
                    ps = ps_big.tile([128, 260], F32, tag="psb")
                    nc.tensor.matmul(ps[:], xt[:, k * 128:(k + 1) * 128],
                                     W0c_sb[:, 0:260], start=True, stop=True)
                    hdst = tw[:, k * T0U:k * T0U + 256]
                    if k % 4 == 3:
                        nc.scalar.copy(out=hdst, in_=ps[:, 0:256])
                    else:
                        nc.vector.tensor_copy(out=hdst, in_=ps[:, 0:256])
                    nc.scalar.copy(
                        out=tw[:, k * T0U + 256:(k + 1) * T0U].bitcast(F32),
                        in_=ps[:, 256:260])
                nc.sync.dma_start(
                    out=table0[blo * 128:(blo + nbm) * 128, :].rearrange(
                        "(b p) u -> p b u", p=128),
                    in_=tw[:, :nbm * T0U].rearrange("p (b u) -> p b u", u=T0U))

            # ---- edge block (shared): tile 0 = self-loops (direct DMA from
            #      per-core rows), tiles 1..KT-1 = indirect gathers ----
            def edge_block(b, table, tunits, H, ad_ap, si_sb, own):
                KH = KT * H
                g = gpool.tile([128, KT * T0U], BF16, tag="g")
                gv = g[:, :KT * tunits]
                nc.sync.dma_start(out=g[:, 0:tunits],
                                  in_=own[b * 128:(b + 1) * 128, :])
                for t in range(1, KT):
                    col = b * KT + t
                    nc.gpsimd.indirect_dma_start(
                        out=g[:, t * tunits:(t + 1) * tunits], out_offset=None,
                        in_=table[:],
                        in_offset=bass.IndirectOffsetOnAxis(
                            ap=si_sb[:, col:col + 1], axis=0),
                    )
                st = stpool.tile([128, KT * 128], BF16, tag="st")
                nc.sync.dma_start(out=st[:],
                                  in_=stD[:, b * KT * 128:(b + 1) * KT * 128])
                sw = swpool.tile([128, KT * 128], BF16, tag="sw")
                nc.sync.dma_start(out=sw[:],
                                  in_=swD[:, b * KT * 128:(b + 1) * KT * 128])
                # per-edge alpha_dst via one-hot gather matmuls -> one PSUM tile
                sev = ps_se.tile([128, KT * H0], F32, tag="pse")
                for t in range(KT):
                    nc.tensor.matmul(sev[:, t * H:(t + 1) * H],
                                     sw[:, t * 128:(t + 1) * 128],
                                     ad_ap, start=True, stop=True)
                # sfull = sev + alpha_src (f32 cols in gathered rows)
                g3f = gv.bitcast(F32).rearrange("p (t u) -> p t u", u=tunits // 2)
                sfull = wpool.tile([128, KT * H0], F32, tag="sfull")
                sf3 = sfull[:, :KH].rearrange("p (t h) -> p t h", h=H)
                nc.vector.tensor_tensor(
                    out=sf3,
                    in0=sev[:, :KH].rearrange("p (t h) -> p t h", h=H),
                    in1=g3f[:, :, 128:128 + H], op=AX.add)
                w1 = wpool.tile([128, KT * H0], F32, tag="w1")
                nc.scalar.activation(w1[:, :KH], sfull[:, :KH], AF.Exp)
                w2 = wpool.tile([128, KT * H0], F32, tag="w2")
                nc.scalar.activation(w2[:, :KH], sfull[:, :KH], AF.Exp, scale=SLOPE)
                wm = wpool.tile([128, KT * H0], BF16, tag="wm")
                nc.vector.tensor_tensor(out=wm[:, :KH], in0=w1[:, :KH],
                                        in1=w2[:, :KH], op=AX.max)
                wm3 = wm[:, :KH].rearrange("p (t h) -> p t h", h=H)
                # rhs = [g*w | w] built with one 4D (or 3D) mult + one copy
                rhs = rpool.tile([128, KT * T0U], BF16, tag="rhs")
                rv = rhs[:, :KT * tunits]
                r3 = rv.rearrange("p (t u) -> p t u", u=tunits)
                if H == 1:
                    nc.vector.tensor_tensor(
                        out=r3[:, :, 0:256],
                        in0=gv.rearrange("p (t u) -> p t u", u=tunits)[:, :, 0:256],
                        in1=wm3.to_broadcast([128, KT, 256]),
                        op=AX.mult)
                else:
                    nc.vector.tensor_tensor(
                        out=r3[:, :, 0:256].rearrange("p t (h c) -> p t h c", c=64),
                        in0=gv.rearrange("p (t u) -> p t u", u=tunits)
                            [:, :, 0:256].rearrange("p t (h c) -> p t h c", c=64),
                        in1=wm3.unsqueeze(3).to_broadcast([128, KT, H, 64]),
                        op=AX.mult)
                nc.vector.tensor_copy(out=r3[:, :, 256:256 + H], in_=wm3)
                # aggregate all tiles into one accumulating PSUM group
                agg = ps_agg.tile([128, 256 + H0], F32, tag="agg")
                aggv = agg[:, :256 + H]
                for t in range(KT):
                    nc.tensor.matmul(aggv, st[:, t * 128:(t + 1) * 128],
                                     rhs[:, t * tunits:t * tunits + 256 + H],
                                     start=(t == 0), stop=(t == KT - 1))
                return agg

            def finish_block(agg, H, bias_sb):
                dn = wpool.tile([128, H0], F32, tag="dn")
                nc.scalar.activation(dn[:, :H], agg[:, 256:256 + H], AF.Copy,
                                     bias=1e-30)
                rc = wpool.tile([128, H0], F32, tag="rc")
                nc.vector.reciprocal(rc[:, :H], dn[:, :H])
                h = apool.tile([128, 256], F32, tag="h")
                if H == 1:
                    nc.vector.tensor_scalar_mul(h[:], agg[:, 0:256], rc[:, 0:1])
                else:
                    nc.vector.tensor_tensor(
                        out=h[:].rearrange("p (a b) -> p a b", b=64),
                        in0=agg[:, 0:256].rearrange("p (a b) -> p a b", b=64),
                        in1=rc[:, :H].unsqueeze(2).to_broadcast([128, H, 64]),
                        op=AX.mult)
                nc.vector.tensor_add(h[:], h[:], bias_sb[:])
                hr = apool.tile([128, 256], BF16, tag="hr")
                nc.scalar.activation(hr[:], h[:], AF.Relu)
                return hr

            # ---- phase E0 + table1 (+ chunked AllGather) ----
            for b in range(BPC):
                agg = edge_block(b, table0, T0U, H0,
                                 ad0_sb[:, b * H0:(b + 1) * H0], si0_sb, t0own)
                hr = finish_block(agg, H0, b0b_sb)
                ps1 = ps_big.tile([128, 512], F32, tag="psb")
                for k in range(2):
                    tps = ps_mlp.tile([128, 128], BF16, tag="psmlp")
                    nc.tensor.transpose(tps[:], hr[:, k * 128:(k + 1) * 128], id_sb[:])
                    c = wpool.tile([128, 128], BF16, tag="hT")
                    nc.vector.tensor_copy(out=c[:], in_=tps[:])
                    nc.tensor.matmul(ps1[:, :258], c[:],
                                     W1c_sb[:, k * 258:(k + 1) * 258],
                                     start=(k == 0), stop=(k == 1))
                t1 = twpool.tile([128, T1U], BF16, tag="t1")
                nc.vector.tensor_copy(out=t1[:, 0:256], in_=ps1[:, 0:256])
                nc.scalar.copy(out=t1[:, 256:258].bitcast(F32), in_=ps1[:, 256:257])
                nc.scalar.copy(out=ad1_sb[:, b:b + 1], in_=ps1[:, 257:258])
                nc.sync.dma_start(out=ag_in[b * 128:(b + 1) * 128, :], in_=t1[:])
                for cch in range(NCHUNK):
                    ready_block = CHB[cch + 1] // BLK - 1
                    if b == ready_block:
                        nc.gpsimd.collective_compute(
                            "AllGather", AX.bypass,
                            replica_groups=[list(range(NCORE))],
                            ins=[ag_in[CHB[cch]:CHB[cch + 1], :]],
                            outs=[table1[NCORE * CHB[cch]:
                                         NCORE * CHB[cch + 1], :]],
                        )

            # ---- phase E1 + batched MLP ----
            gstart = 0
            while gstart < BPC:
                G = min(MLPG, BPC - gstart)
                h2T0 = hpool.tile([128, 128 * MLPG], BF16, tag="h2T0")
                h2T1 = hpool.tile([128, 128 * MLPG], BF16, tag="h2T1")
                for gi in range(G):
                    b = gstart + gi
                    agg = edge_block(b, table1, T1U, 1, ad1_sb[:, b:b + 1],
                                     si1_sb, ag_in)
                    h2 = finish_block(agg, 1, b1b_sb)
                    for k, wt in enumerate((h2T0, h2T1)):
                        tps = ps_mlp.tile([128, 128], BF16, tag="psmlp")
                        nc.tensor.transpose(tps[:], h2[:, k * 128:(k + 1) * 128], id_sb[:])
                        nc.vector.tensor_copy(out=wt[:, gi * 128:(gi + 1) * 128], in_=tps[:])
                NW = 128 * G
                h2T = (h2T0, h2T1)
                a1T = []
                for oc in range(4):
                    ps = ps_mlp.tile([128, 128 * MLPG], F32, tag="psmlp")
                    for kc in range(2):
                        nc.tensor.matmul(
                            ps[:, :NW],
                            lw1_sb[:, kc * 512 + oc * 128:kc * 512 + (oc + 1) * 128],
                            h2T[kc][:, :NW], start=(kc == 0), stop=(kc == 1))
                    a = a1pool.tile([128, 128 * MLPG], BF16, tag="a1T")
                    nc.scalar.activation(a[:, :NW], ps[:, :NW], AF.Relu,
                                         bias=lb1_sb[:, oc:oc + 1])
                    a1T.append(a)
                a2T = []
                for oc in range(8):
                    ps = ps_mlp.tile([128, 128 * MLPG], F32, tag="psmlp")
                    for kc in range(4):
                        nc.tensor.matmul(
                            ps[:, :NW],
                            lw2_sb[:, kc * 1024 + oc * 128:kc * 1024 + (oc + 1) * 128],
                            a1T[kc][:, :NW], start=(kc == 0), stop=(kc == 3))
                    a = a2pool.tile([128, 128 * MLPG], BF16, tag="a2T")
                    nc.scalar.activation(a[:, :NW], ps[:, :NW], AF.Relu,
                                         bias=lb2_sb[:, oc:oc + 1])
                    a2T.append(a)
                # layer 3 transposed: logits^T [10, NW] via 8 wide matmuls,
                # bias+relu with per-partition bias, then tiny PE transposes back
                psL = ps_mlp.tile([128, 128 * MLPG], F32, tag="psmlp")
                for kc in range(8):
                    nc.tensor.matmul(psL[0:10, :NW],
                                     lw3_sb[:, kc * 10:(kc + 1) * 10],
                                     a2T[kc][:, :NW],
                                     start=(kc == 0), stop=(kc == 7))
                lgT = a1pool.tile([128, 128 * MLPG], F32, tag="lgT", bufs=2)
                nc.scalar.activation(lgT[0:10, :NW], psL[0:10, :NW], AF.Relu,
                                     bias=lb3_sb[0:10, 0:1])
                for gi in range(G):
                    b = gstart + gi
                    tps3 = ps_se.tile([128, 16], F32, tag="pse")
                    nc.tensor.transpose(tps3[:, 0:10],
                                        lgT[0:10, gi * 128:(gi + 1) * 128],
                                        idf_sb[0:10, 0:10])
                    elg = wpool.tile([128, 16], F32, tag="elg")
                    nc.scalar.activation(elg[:, :10], tps3[:, 0:10], AF.Exp)
                    ssum = wpool.tile([128, 1], F32, tag="ssum")
                    nc.vector.reduce_sum(out=ssum[:], in_=elg[:, :10],
                                         axis=mybir.AxisListType.X)
                    rcs = wpool.tile([128, 1], F32, tag="rcs")
                    nc.vector.reciprocal(rcs[:], ssum[:])
                    ob = wpool.tile([128, 16], F32, tag="ob")
                    nc.vector.tensor_scalar_mul(ob[:, :10], elg[:, :10], rcs[:])
                    nc.sync.dma_start(out=out[b * 128:(b + 1) * 128, :], in_=ob[:, :10])
                gstart += G

    nc.compile()
    return nc


# ======== host-side prep ========
import heapq

def build_graph(edge_index):
    src, dst = edge_index[0].astype(np.int64), edge_index[1].astype(np.int64)
    keep = src != dst
    src, dst = src[keep], dst[keep]
    loops = np.arange(50000, dtype=np.int64)
    return np.concatenate([src, loops]), np.concatenate([dst, loops])


def assign_blocks(dst, EPB):
    N = 50000
    deg = np.bincount(dst, minlength=N)
    order = np.argsort(-deg, kind='stable')
    nblocks = NCORE * BPC
    heap = [(0, b) for b in range(nblocks)]
    heapq.heapify(heap)
    slots_used = np.zeros(nblocks, np.int64)
    load = np.zeros(nblocks, np.int64)
    block_of = np.empty(N, np.int64)
    deferred = []
    for v in order:
        d = deg[v]
        while True:
            if not heap:
                return None, None, load
            l, b = heapq.heappop(heap)
            if (slots_used[b] < BLK and load[b] + d <= EPB
                    and (load[b] + d) - (slots_used[b] + 1) <= EPB - BLK):
                block_of[v] = b
                slots_used[b] += 1
                load[b] += d
                heapq.heappush(heap, (load[b], b))
                break
            deferred.append(b)
        for bb in deferred:
            if slots_used[bb] < BLK:
                heapq.heappush(heap, (load[bb], bb))
        deferred.clear()
    perm = np.full(NPAD, -1, np.int64)
    newid = np.empty(N, np.int64)
    fill = np.zeros(nblocks, np.int64)
    for v in range(N):
        b = block_of[v]
        s = b * BLK + fill[b]
        fill[b] += 1
        perm[s] = v
        newid[v] = s
    return perm, newid, load


def chunk_rowmap():
    s = np.arange(NPAD)
    core = s // VLOC
    pos = s % VLOC
    chb = np.asarray(CHB)
    ch = np.searchsorted(chb, pos, side='right') - 1
    width = chb[ch + 1] - chb[ch]
    return NCORE * chb[ch] + core * width + (pos - chb[ch])


def build_streams(src, dst, newid, KT):
    EPB = KT * 128
    nsrc, ndst = newid[src], newid[dst]
    blk = ndst // BLK
    order = np.argsort(blk, kind='stable')
    nsrc_o, ndst_o, blk_o = nsrc[order], ndst[order], blk[order]
    counts = np.bincount(blk_o, minlength=NCORE * BPC)
    assert counts.max() <= EPB
    starts = np.concatenate([[0], np.cumsum(counts)[:-1]])
    # self-loops -> tile 0 at slot = own position; non-self -> slots 128+rank
    selfm = nsrc_o == ndst_o
    cns = np.cumsum(~selfm)
    base = np.concatenate([[0], cns])[starts[blk_o]]
    wns = cns - 1 - base
    within = np.where(selfm, ndst_o % BLK, BLK + wns)
    assert within.max() < EPB
    cmap = chunk_rowmap()
    src0 = np.zeros((NCORE * BPC, EPB), np.int32)
    src1 = np.zeros((NCORE * BPC, EPB), np.int32)
    dstloc = np.full((NCORE * BPC, EPB), -1, np.int64)
    src0[blk_o, within] = nsrc_o.astype(np.int32)
    src1[blk_o, within] = cmap[nsrc_o].astype(np.int32)
    dstloc[blk_o, within] = ndst_o % BLK
    src0 = src0.reshape(NCORE, BPC * KT, 128)
    src1 = src1.reshape(NCORE, BPC * KT, 128)
    dstloc = dstloc.reshape(NCORE, BPC * KT, 128)
    bf = ml_dtypes.bfloat16
    s0 = [np.ascontiguousarray(src0[c].T) for c in range(NCORE)]
    s1 = [np.ascontiguousarray(src1[c].T) for c in range(NCORE)]
    stA, swA = [], []
    iot = np.arange(128, dtype=np.int64)
    for c in range(NCORE):
        oh = (dstloc[c][:, :, None] == iot[None, None, :])   # [col, e, n]
        stA.append(np.ascontiguousarray(
            oh.transpose(1, 0, 2).reshape(128, BPC * KT * 128).astype(bf)))
        swA.append(np.ascontiguousarray(
            oh.transpose(2, 0, 1).reshape(128, BPC * KT * 128).astype(bf)))
    return s0, s1, stA, swA


def prep_inputs(inputs, KT):
    bf = ml_dtypes.bfloat16
    x = np.asarray(inputs['x'], np.float32)
    src, dst = build_graph(np.asarray(inputs['edge_index']))
    perm, newid, load = assign_blocks(dst, KT * 128)
    assert perm is not None, f"packing failed at KT={KT}, max load {load.max()}"
    s0, s1, stA, swA = build_streams(src, dst, newid, KT)
    W0 = np.asarray(inputs['W0'], np.float32)
    W1 = np.asarray(inputs['W1'], np.float32)
    a_src0 = np.asarray(inputs['a_src0'], np.float32)
    a_dst0 = np.asarray(inputs['a_dst0'], np.float32)
    a_src1 = np.asarray(inputs['a_src1'], np.float32).reshape(256, 1)
    a_dst1 = np.asarray(inputs['a_dst1'], np.float32).reshape(256, 1)
    # head-major layout: table0 col j = head j//64, channel j%64  (== W0 order)
    jj = np.arange(256)
    hh, cc = jj // 64, jj % 64
    A0s = np.zeros((256, H0), np.float32)
    A0d = np.zeros((256, H0), np.float32)
    A0s[jj, hh] = a_src0[hh, cc]
    A0d[jj, hh] = a_dst0[hh, cc]
    W0cat = np.concatenate([W0, W0 @ A0s, W0 @ A0d], 1).astype(bf)   # [128, 264]
    W1cat = np.concatenate([W1, W1 @ a_src1, W1 @ a_dst1], 1)        # [256, 258]
    W1catP = np.ascontiguousarray(
        W1cat.reshape(2, 128, 258).transpose(1, 0, 2).reshape(128, 2 * 258)).astype(bf)
    lw1 = np.asarray(inputs['lw1'], np.float32)
    lw2 = np.asarray(inputs['lw2'], np.float32)
    lw3 = np.asarray(inputs['lw3'], np.float32)
    lw1P = np.ascontiguousarray(lw1.reshape(2, 128, 512).transpose(1, 0, 2).reshape(128, 1024)).astype(bf)
    lw2P = np.ascontiguousarray(lw2.reshape(4, 128, 1024).transpose(1, 0, 2).reshape(128, 4096)).astype(bf)
    lw3P = np.ascontiguousarray(lw3.reshape(8, 128, 10).transpose(1, 0, 2).reshape(128, 80)).astype(bf)
    lb1c = np.ascontiguousarray(np.asarray(inputs['lb1'], np.float32).reshape(4, 128).T)
    lb2c = np.ascontiguousarray(np.asarray(inputs['lb2'], np.float32).reshape(8, 128).T)
    lb3col = np.zeros((128, 1), np.float32)
    lb3col[:10, 0] = np.asarray(inputs['lb3'], np.float32)
    b0b = np.tile(np.asarray(inputs['b0'], np.float32), (128, 1)).astype(np.float32)
    b1b = np.tile(np.asarray(inputs['b1'], np.float32), (128, 1)).astype(np.float32)
    ident = np.eye(128, dtype=np.float32).astype(bf)
    identf = np.eye(16, dtype=np.float32)

    real = perm >= 0
    xp = np.zeros((NPAD, 128), np.float32)
    xp[real] = x[perm[real]]
    xT_full = np.ascontiguousarray(xp.T).astype(bf)

    common = dict(xT=xT_full, W0cat=W0cat, W1catP=W1catP, lw1P=lw1P, lw2P=lw2P,
                  lw3P=lw3P, lb1c=lb1c, lb2c=lb2c, lb3col=lb3col, b0b=b0b,
                  b1b=b1b, ident=ident, identf=identf)
    in_maps = []
    for c in range(NCORE):
        m = dict(common)
        m['x_ownT'] = np.ascontiguousarray(xp[c * VLOC:(c + 1) * VLOC].T).astype(bf)
        m['srcidx0'] = s0[c]
        m['srcidx1'] = s1[c]
        m['stD'] = stA[c]
        m['swD'] = swA[c]
        in_maps.append(m)
    return in_maps, perm


def pick_KT(inputs):
    src, dst = build_graph(np.asarray(inputs['edge_index']))
    for KT in (9, 10, 11):
        perm, newid, load = assign_blocks(dst, KT * 128)
        if perm is not None:
            return KT
    raise RuntimeError("packing failed")


def assemble_output(results, perm):
    full = np.concatenate([results[c]['out'] for c in range(NCORE)], 0)
    out = np.zeros((50000, 10), np.float32)
    real = perm >= 0
    out[perm[real]] = full[real]
    return out


# ======== public entrypoint ========
_CACHE = {}


def _get_nc(KT):
    if ('nc', KT) not in _CACHE:
        _CACHE[('nc', KT)] = build_nc(KT)
    return _CACHE[('nc', KT)]


def run(inputs, trace=False, trace_cores=None, tmpdir=None):
    from concourse.bass_utils import run_bass_kernel_spmd
    KT = pick_KT(inputs)
    nc = _get_nc(KT)
    in_maps, perm = prep_inputs(inputs, KT)
    res = run_bass_kernel_spmd(nc, in_maps, list(range(NCORE)),
                               trace=trace, trace_cores=trace_cores,
                               tmpdir=tmpdir)
    return assemble_output(res.results, perm), res


def kernel(**inputs):
    out, _ = run(inputs, trace=False)
    return out
